# revision 1
# baseline (speedup 1.0000x reference)
"""MultiHeadAttention kernel for 8 Trainium2 NeuronCores.

Reference semantics (note the *direct* reshape to [B, H, T, hs], which makes
"heads" contiguous 256-row blocks of Y.reshape(1536, 64) where Y = x[b] @ W):

    k = (x @ Wk).reshape(B, H, T, hs); q, v likewise
    wei = softmax(mask(q @ k^T * C**-0.5))        (causal over chunk index)
    out = (wei @ v).reshape(B, T, C) @ Wp + bp

Sharding: data-parallel over batch — 16 batches per core, weights replicated,
no collectives.

Per-core dataflow (B=16 local batches, T=256, C=384, H=6, hs=64):
  1. QKV: YqT/YkT = Wq/Wk^T x^T computed in PSUM [c-tile, t], evicted with a
     stride-6 interleave into Zq/Zk [64, 1536] so head slices are plain APs.
     Yv computed in the natural [t, c] orientation and round-tripped through
     DRAM to reshape into chunk-row tiles V [128, 12*64].
  2. Attention per head: S^T = K^T.T @ Q^T (PSUM), exp via ACT (scale fused),
     causal mask via GPSIMD multiply, softmax denominator via ones-matmul
     over partitions, reciprocal on DVE, broadcast via K=1 matmul, PV matmul
     with natural-layout V, normalize on DVE into OcT [64, 1536].
  3. Projection: Z = sum_j OmatT_j.T @ Wp_j with OmatT_j a stride-6 slice of
     OcT; bias added during PSUM eviction; contiguous DMA to the output.

Every matmul operand is float32r (TF32-like fast mode, 1 cycle/row at free
dim >= 256); the walrus verifier requires those tensors to be declared f32r
at their producers, so all matmul-feeding tiles/DRAM tensors are f32r.
"""

import sys

if "/opt/trn_rl_repo" not in sys.path:
    sys.path.insert(0, "/opt/trn_rl_repo")

import numpy as np

import concourse.bass as bass
import concourse.mybir as mybir
import concourse.tile as tile
from concourse import bacc
from concourse.bass_utils import run_bass_kernel_spmd

F32 = mybir.dt.float32
F32R = mybir.dt.float32r
Exp = mybir.ActivationFunctionType.Exp

N_CORES = 8
B, T, C = 128, 256, 384
H, HS = 6, 64
NB = B // N_CORES          # batches per core
SCALE = C ** (-0.5)


def build_program(trace_sim=False, sim_init=False):
    nc = bacc.Bacc("TRN2", target_bir_lowering=False, debug=False)

    xT_d = nc.dram_tensor("xT", [NB, 3, 128, T], F32R, kind="ExternalInput")
    wq_d = nc.dram_tensor("wq", [C, C], F32R, kind="ExternalInput")
    wk_d = nc.dram_tensor("wk", [C, C], F32R, kind="ExternalInput")
    wv_d = nc.dram_tensor("wv", [C, C], F32R, kind="ExternalInput")
    wp_d = nc.dram_tensor("wp", [C, C], F32R, kind="ExternalInput")
    bpb_d = nc.dram_tensor("bpb", [128, C], F32, kind="ExternalInput")
    # masks[:, 0:128] = upper-tri (s<=t); masks[:, 128:384] = [zeros | upper-tri]
    mask_d = nc.dram_tensor("masks", [128, C + 12], F32R, kind="ExternalInput")
    ones_d = nc.dram_tensor("ones128", [128, 1], F32R, kind="ExternalInput")
    onesr_d = nc.dram_tensor("ones64", [1, 64], F32R, kind="ExternalInput")
    vsc_d = nc.dram_tensor("vsc", [NB, T, C], F32R)
    out_d = nc.dram_tensor("out", [NB, T, C], F32, kind="ExternalOutput")

    with tile.TileContext(nc, trace_sim=trace_sim) as tc:
        with (
            tc.tile_pool(name="const", bufs=1) as cst,
            tc.tile_pool(name="xt", bufs=3) as xtp,
            tc.tile_pool(name="zqk", bufs=2) as zqkp,
            tc.tile_pool(name="yv", bufs=3) as yvp,
            tc.tile_pool(name="vsb", bufs=2) as vp,
            tc.tile_pool(name="ee", bufs=14) as ep,
            tc.tile_pool(name="rsi", bufs=4) as rsp,
            tc.tile_pool(name="oct", bufs=2) as octp,
            tc.tile_pool(name="zo", bufs=3) as zop,
            tc.tile_pool(name="psA", bufs=2, space="PSUM") as psA,
            tc.tile_pool(name="psB", bufs=4, space="PSUM") as psB,
            tc.tile_pool(name="psBC", bufs=2, space="PSUM") as psBC,
        ):
            # ---- constants ----
            wq_sb = cst.tile([128, 3, C], F32R, tag="wq")
            wk_sb = cst.tile([128, 3, C], F32R, tag="wk")
            wv_sb = cst.tile([128, 3, C], F32R, tag="wv")
            wp_sb = cst.tile([64, 6, C], F32R, tag="wp")
            for wsb, wd in ((wq_sb, wq_d), (wk_sb, wk_d), (wv_sb, wv_d)):
                nc.sync.dma_start(wsb[:], wd.rearrange("(k p) c -> p k c", p=128))
            nc.sync.dma_start(wp_sb[:], wp_d.rearrange("(j p) c -> p j c", p=64))
            bpb = cst.tile([128, C], F32, tag="bpb")
            nc.sync.dma_start(bpb[:], bpb_d[:])
            masks = cst.tile([128, C + 12], F32R, tag="masks")
            nc.sync.dma_start(masks[:], mask_d[:])
            ones128 = cst.tile([128, 1], F32R, tag="ones128")
            nc.sync.dma_start(ones128[:], ones_d[:])
            ones64 = cst.tile([1, 64], F32R, tag="ones64")
            nc.sync.dma_start(ones64[:], onesr_d[:])

            for b in range(NB):
                # ---------- stage 1: QKV projections (batch pairs) ----------
                if b % 2 == 0:
                    xt = xtp.tile([128, 3, 2 * T], F32R, tag="xt")
                    for n in range(2):
                        nc.sync.dma_start(
                            xt[:].rearrange("p k (n t) -> p k n t", n=2)
                                 [:, :, n, :],
                            xT_d[b + n].rearrange("k p t -> p k t"))
                    zq2 = zqkp.tile([64, 12 * T], F32R, tag="zq")
                    zk2 = zqkp.tile([64, 12 * T], F32R, tag="zk")
                    if sim_init:
                        nc.vector.memset(zq2[:], 0.0)
                        nc.vector.memset(zk2[:], 0.0)
                    for wsb, z2 in ((wq_sb, zq2), (wk_sb, zk2)):
                        for m in range(3):
                            pq = psB.tile([128, 2 * T], F32, tag="mm")
                            for k in range(3):
                                nc.tensor.matmul(
                                    pq[:],
                                    wsb[:, k, m * 128:(m + 1) * 128],
                                    xt[:, k, :],
                                    start=(k == 0), stop=(k == 2),
                                )
                            # eviction: [d, bb*1536 + 6t + j] for bb in {0,1}
                            nc.vector.tensor_copy(
                                z2[:].rearrange("d (n f) -> d n f", n=2)
                                     [:, :, 2 * m:6 * T:6]
                                     .rearrange("d n t -> d (n t)"),
                                pq[0:64, :])
                            nc.scalar.copy(
                                z2[:].rearrange("d (n f) -> d n f", n=2)
                                     [:, :, 2 * m + 1:6 * T:6]
                                     .rearrange("d n t -> d (n t)"),
                                pq[64:128, :])
                    _pair = (xt, zq2, zk2)
                else:
                    xt, zq2, zk2 = _pair
                zq = zq2[:, (b % 2) * 6 * T:(b % 2 + 1) * 6 * T]
                zk = zk2[:, (b % 2) * 6 * T:(b % 2 + 1) * 6 * T]

                for m in range(2):
                    pv = psA.tile([128, C], F32, tag="mmA")
                    for k in range(3):
                        nc.tensor.matmul(
                            pv[:],
                            xt[:, k, (b % 2) * T + m * 128:
                               (b % 2) * T + (m + 1) * 128],
                            wv_sb[:, k, :],
                            start=(k == 0), stop=(k == 2),
                        )
                    yv = yvp.tile([128, C], F32R, tag="yv")
                    nc.vector.tensor_copy(yv[:], pv[:])
                    nc.sync.dma_start(vsc_d[b, m * 128:(m + 1) * 128, :], yv[:])
                # V with a ones column appended per 64-col group: [128, 12*65]
                v_sb = vp.tile([128, 12 * (HS + 1)], F32R, tag="vsb")
                if sim_init:
                    nc.vector.memset(v_sb[:], 0.0)
                nc.vector.tensor_copy(v_sb[:, HS::HS + 1], masks[:, C:C + 12])
                nc.sync.dma_start(
                    v_sb[:].rearrange("p (g d) -> p g d", d=HS + 1)[:, :, 0:HS],
                    vsc_d[b].rearrange("t c -> (t c)")
                            .rearrange("(g p d) -> p g d", p=128, d=64),
                )

                # ---------- stage 2: attention ----------
                e_tiles = []
                for h in range(H):
                    qs = zq[:, h * T:(h + 1) * T]
                    e01 = []
                    for half in range(2):
                        st = psB.tile([128, T], F32, tag="mm")
                        nc.tensor.matmul(
                            st[:],
                            zk[:, h * T + 128 * half:h * T + 128 * (half + 1)],
                            qs,
                            start=True, stop=True,
                        )
                        e = ep.tile([128, T], F32R, tag="e")
                        nc.scalar.activation(e[:], st[:], Exp, scale=SCALE)
                        if half == 0:
                            # diag block: mask s' <= t' on cols 0:128
                            nc.gpsimd.tensor_mul(
                                e[:, 0:128], e[:, 0:128], masks[:, 0:128])
                        else:
                            # lower half: zero cols 0:128, tri cols 128:256
                            nc.gpsimd.tensor_mul(e[:], e[:], masks[:, 128:384])
                        e01.append(e)
                    e_tiles.append(e01)

                # PV with fused rowsum (row 64 of each lhsT is ones)
                po_tiles = []
                rsi_tiles = []
                for h in range(H):
                    e0, e1 = e_tiles[h]
                    po = psB.tile([HS + 1, T], F32, tag="mm")
                    nc.tensor.matmul(
                        po[:], v_sb[:, (2 * h) * (HS + 1):(2 * h) * (HS + 1) + HS + 1],
                        e0[:], start=True, stop=False)
                    nc.tensor.matmul(
                        po[:],
                        v_sb[:, (2 * h + 1) * (HS + 1):(2 * h + 1) * (HS + 1) + HS + 1],
                        e1[:], start=False, stop=True)
                    po_tiles.append(po)
                    if h % 2 == 0:
                        rsi = rsp.tile([1, 2 * T], F32R, tag="rsi")
                        rsi_tiles.append(rsi)
                    with nc.allow_low_precision(reason="softmax denom in f32r"):
                        nc.vector.reciprocal(
                            rsi_tiles[h // 2][:, (h % 2) * T:(h % 2 + 1) * T],
                            po[HS:HS + 1, :])

                if True:
                    ocT = octp.tile([64, 6 * T], F32R, tag="ocT")
                    if sim_init:
                        nc.vector.memset(ocT[:], 0.0)
                for hp in range(H // 2):
                    bc = psBC.tile([64, 2 * T], F32, tag="bc")
                    nc.tensor.matmul(bc[:], ones64[:], rsi_tiles[hp][:],
                                     start=True, stop=True)
                    bc_sb = rsp.tile([64, 2 * T], F32, tag="bcsb")
                    nc.scalar.copy(bc_sb[:], bc[:])
                    for hh in range(2):
                        h = 2 * hp + hh
                        nc.vector.tensor_mul(
                            ocT[:, h * T:(h + 1) * T],
                            po_tiles[h][0:HS, :],
                            bc_sb[:, hh * T:(hh + 1) * T])

                # ---------- stage 3: output projection ----------
                for m in range(2):
                    pz = psA.tile([128, C], F32, tag="mmA")
                    for j in range(6):
                        nc.tensor.matmul(
                            pz[:],
                            ocT[:, 768 * m + j:768 * (m + 1):6],
                            wp_sb[:, j, :],
                            start=(j == 0), stop=(j == 5),
                        )
                    zo = zop.tile([128, C], F32, tag="zo")
                    nc.vector.tensor_add(zo[:], pz[:], bpb[:])
                    nc.sync.dma_start(out_d[b, m * 128:(m + 1) * 128, :], zo[:])

    nc.compile()
    return nc


def make_in_maps(x, Wk, Wq, Wv, Wp, bp):
    ut = (np.arange(128)[:, None] <= np.arange(128)[None, :])
    masks = np.zeros((128, C + 12), np.float32)
    masks[:, 0:128] = ut
    masks[:, 256:384] = ut            # [zeros | tri] occupying cols 128:384
    masks[:, 384:396] = 1.0           # ones columns for the fused-rowsum V
    common = dict(
        wq=np.ascontiguousarray(Wq, np.float32),
        wk=np.ascontiguousarray(Wk, np.float32),
        wv=np.ascontiguousarray(Wv, np.float32),
        wp=np.ascontiguousarray(Wp, np.float32),
        bpb=np.broadcast_to(np.asarray(bp, np.float32), (128, C)).copy(),
        masks=masks,
        ones128=np.ones((128, 1), np.float32),
        ones64=np.ones((1, 64), np.float32),
    )
    in_maps = []
    for c in range(N_CORES):
        xs = np.asarray(x[c * NB:(c + 1) * NB], np.float32)
        xT = np.ascontiguousarray(xs.transpose(0, 2, 1)).reshape(NB, 3, 128, T)
        in_maps.append(dict(common, xT=xT))
    return in_maps


_CACHE = {}


def kernel(x, Wk, Wq, Wv, Wp, bp, _trace=False, _tmpdir=None):
    if "nc" not in _CACHE:
        _CACHE["nc"] = build_program()
    nc = _CACHE["nc"]
    in_maps = make_in_maps(x, Wk, Wq, Wv, Wp, bp)
    res = run_bass_kernel_spmd(nc, in_maps, list(range(N_CORES)),
                               trace=_trace, tmpdir=_tmpdir)
    _CACHE["last_results"] = res
    out = np.concatenate([np.asarray(r["out"]) for r in res.results], axis=0)
    return out.reshape(B, T, C).astype(np.float32)

